# revision 3
# baseline (speedup 1.0000x reference)
"""Equivariant block-diagonal linear (irreps 256x0e + 256x1o + 128x2e) on 8
Trainium2 NeuronCores.

Math: for each irrep segment (mul, ird), out[b, v, i] = c * sum_u w[u,v] *
x[b, u, i] with c = 1/sqrt(mul). x columns are laid out mul-major:
col = seg_off + u*ird + i.

Data-parallel sharding: batch 50000 -> 8 cores x 6272 rows (padded).
Per core, per 128-row batch tile:
  1. DMA the [128, 1664] x tile to SBUF (contiguous).
  2. For each (segment, i, u-chunk): PE-transpose the strided column slice
     x[:, off+i::ird] (128 u values) into PSUM -> DVE copy to SBUF. This
     yields xT [u, b] tiles, de-interleaved by i.
  3. matmul(psum[b, v] += xT[u_chunk].T @ w[u_chunk, :]) accumulating over
     u-chunks; weights stay SBUF-resident in natural [u, v] layout,
     pre-scaled by c on the host.
  4. DVE copy psum -> strided columns of the [128, 1664] output staging
     tile; one contiguous DMA per batch tile to DRAM.
"""

import numpy as np

N_CORES = 8
BATCH = 50000
X_DIM = 1664
P = 128
ROWS_PER_CORE = 6272  # 49 tiles of 128; 8*6272 = 50176 >= 50000
BT = ROWS_PER_CORE // P

# (mul, ird, x/y col offset, n u-chunks of 128)
SEGS = [
    (256, 1, 0, 2),
    (256, 3, 256, 2),
    (128, 5, 1024, 1),
]

_cache = {}


def _build_program(mm_dtype_name: str):
    import concourse.bacc as bacc
    import concourse.mybir as mybir
    from concourse.tile import TileContext

    mm_dt = getattr(mybir.dt, mm_dtype_name)

    nc = bacc.Bacc(
        "TRN2", target_bir_lowering=False, debug=False, num_devices=N_CORES
    )
    x = nc.dram_tensor("x", [ROWS_PER_CORE, X_DIM], mybir.dt.float32,
                       kind="ExternalInput")
    w1 = nc.dram_tensor("w1", [256, 256], mybir.dt.float32, kind="ExternalInput")
    w2 = nc.dram_tensor("w2", [256, 256], mybir.dt.float32, kind="ExternalInput")
    w3 = nc.dram_tensor("w3", [128, 128], mybir.dt.float32, kind="ExternalInput")
    ident = nc.dram_tensor("ident", [P, P], mybir.dt.float32, kind="ExternalInput")
    y = nc.dram_tensor("y", [ROWS_PER_CORE, X_DIM], mybir.dt.float32,
                       kind="ExternalOutput")
    w_dram = [w1, w2, w3]

    with TileContext(nc) as tc:
        with (
            tc.tile_pool(name="wpool", bufs=1) as wpool,
            tc.tile_pool(name="xin", bufs=3) as xin,
            tc.tile_pool(name="xtp", bufs=4, space="PSUM") as xtp,
            tc.tile_pool(name="xts", bufs=16) as xts,
            tc.tile_pool(name="pop", bufs=4, space="PSUM") as pop,
            tc.tile_pool(name="outp", bufs=3) as outp,
        ):
            # Resident weights: per segment, u-chunk tiles [128, mul].
            w_sb = []
            for si, (mul, ird, off, n_uc) in enumerate(SEGS):
                chunks = []
                for uc in range(n_uc):
                    t = wpool.tile([P, mul], mm_dt, tag=f"w{si}_{uc}")
                    eng = nc.gpsimd if mm_dt != mybir.dt.float32 else nc.sync
                    eng.dma_start(out=t[:], in_=w_dram[si][uc * P:(uc + 1) * P, :])
                    chunks.append(t)
                w_sb.append(chunks)
            ident_sb = wpool.tile([P, P], mybir.dt.float32, tag="ident")
            nc.sync.dma_start(out=ident_sb[:], in_=ident[:, :])

            for bt in range(BT):
                r0 = bt * P
                xt = xin.tile([P, X_DIM], mybir.dt.float32, tag="x")
                nc.sync.dma_start(out=xt[:], in_=x[r0:r0 + P, :])
                ot = outp.tile([P, X_DIM], mybir.dt.float32, tag="o")

                for (mul, ird, off, n_uc) in SEGS:
                    for i in range(ird):
                        xs_chunks = []
                        for uc in range(n_uc):
                            start = off + uc * P * ird + i
                            src = xt[:, start:start + ird * (P - 1) + 1:ird]
                            tp = xtp.tile([P, P], mybir.dt.float32, tag="tp")
                            nc.tensor.transpose(tp[:], src, ident_sb[:])
                            xs = xts.tile([P, P], mm_dt, tag="xs")
                            nc.vector.tensor_copy(out=xs[:], in_=tp[:])
                            xs_chunks.append(xs)
                        po = pop.tile([P, mul], mybir.dt.float32, tag="po")
                        for uc in range(n_uc):
                            nc.tensor.matmul(
                                po[:],
                                xs_chunks[uc][:],
                                w_sb[SEGS.index((mul, ird, off, n_uc))][uc][:],
                                start=(uc == 0),
                                stop=(uc == n_uc - 1),
                            )
                        dst = ot[:, off + i:off + i + ird * (mul - 1) + 1:ird]
                        nc.vector.tensor_copy(out=dst, in_=po[:])

                nc.sync.dma_start(out=y[r0:r0 + P, :], in_=ot[:])

    nc.compile()
    return nc


def _get_program(mm_dtype_name: str):
    if mm_dtype_name not in _cache:
        _cache[mm_dtype_name] = _build_program(mm_dtype_name)
    return _cache[mm_dtype_name]


def kernel(x: np.ndarray, weight: np.ndarray, _mm_dtype: str = "float32",
           _trace: bool = False):
    from concourse.bass_utils import run_bass_kernel_spmd

    nc = _get_program(_mm_dtype)

    x = np.ascontiguousarray(np.asarray(x, dtype=np.float32))
    weight = np.asarray(weight, dtype=np.float32)

    w1 = (weight[:65536].reshape(256, 256) / np.sqrt(np.float32(256.0))).astype(np.float32)
    w2 = (weight[65536:131072].reshape(256, 256) / np.sqrt(np.float32(256.0))).astype(np.float32)
    w3 = (weight[131072:].reshape(128, 128) / np.sqrt(np.float32(128.0))).astype(np.float32)
    ident = np.eye(P, dtype=np.float32)

    xp = np.zeros((N_CORES * ROWS_PER_CORE, X_DIM), dtype=np.float32)
    xp[:BATCH] = x

    in_maps = [
        {
            "x": xp[c * ROWS_PER_CORE:(c + 1) * ROWS_PER_CORE],
            "w1": w1,
            "w2": w2,
            "w3": w3,
            "ident": ident,
        }
        for c in range(N_CORES)
    ]
    res = run_bass_kernel_spmd(
        nc, in_maps, list(range(N_CORES)), trace=_trace
    )
    out = np.concatenate([res.results[c]["y"] for c in range(N_CORES)], axis=0)
    if _trace:
        kernel.last_exec_time_ns = res.exec_time_ns
    return out[:BATCH]


# revision 8
# speedup vs baseline: 1.2024x; 1.2024x over previous
"""Equivariant block-diagonal linear (irreps 256x0e + 256x1o + 128x2e) on 8
Trainium2 NeuronCores.

Math: for each irrep segment (mul, ird), out[b, v, i] = c * sum_u w[u,v] *
x[b, u, i] with c = 1/sqrt(mul). x columns are laid out mul-major:
col = seg_off + u*ird + i.

Data-parallel sharding: batch 50000 -> 8 cores x 6272 rows (padded).
Per core, per 128-row batch tile:
  1. DMA the [128, 1664] x tile to SBUF (contiguous).
  2. For each (segment, i, u-chunk): PE-transpose the strided column slice
     x[:, off+i::ird] (128 u values) into PSUM -> DVE copy to SBUF. This
     yields xT [u, b] tiles, de-interleaved by i.
  3. matmul(psum[b, v] += xT[u_chunk].T @ w[u_chunk, :]) accumulating over
     u-chunks; weights stay SBUF-resident in natural [u, v] layout,
     pre-scaled by c on the host.
  4. DVE copy psum -> strided columns of the [128, 1664] output staging
     tile; one contiguous DMA per batch tile to DRAM.
"""

import numpy as np

N_CORES = 8
BATCH = 50000
X_DIM = 1664
P = 128
ROWS_PER_CORE = 6272  # 49 tiles of 128; 8*6272 = 50176 >= 50000
BT = ROWS_PER_CORE // P

# (mul, ird, x/y col offset, n u-chunks of 128)
SEGS = [
    (256, 1, 0, 2),
    (256, 3, 256, 2),
    (128, 5, 1024, 1),
]

_cache = {}


def _build_program(mm_dtype_name: str):
    import concourse.bacc as bacc
    import concourse.mybir as mybir
    from concourse.tile import TileContext

    use_f32r = mm_dtype_name == "float32r"
    f32 = mybir.dt.float32
    f32r = mybir.dt.float32r

    # In f32r mode, pad seg3's weight to N=256 so the matmul free dim
    # reaches the 1-cycle/row threshold (ap_size >= 256).
    w3_cols = 256 if use_f32r else 128
    mm_cols = [256, 256, w3_cols]

    nc = bacc.Bacc(
        "TRN2", target_bir_lowering=False, debug=False, num_devices=N_CORES
    )
    x = nc.dram_tensor("x", [ROWS_PER_CORE, X_DIM], f32, kind="ExternalInput")
    w1 = nc.dram_tensor("w1", [256, 256], f32, kind="ExternalInput")
    w2 = nc.dram_tensor("w2", [256, 256], f32, kind="ExternalInput")
    w3 = nc.dram_tensor("w3", [128, 256], f32, kind="ExternalInput")
    ident = nc.dram_tensor("ident", [P, P], f32, kind="ExternalInput")
    y = nc.dram_tensor("y", [ROWS_PER_CORE, X_DIM], f32, kind="ExternalOutput")
    w_dram = [w1, w2, w3]

    mm_dt = f32r if use_f32r else f32

    with TileContext(nc) as tc:
        with (
            tc.tile_pool(name="wpool", bufs=1) as wpool,
            tc.tile_pool(name="xin", bufs=3) as xin,
            tc.tile_pool(name="xtp", bufs=4, space="PSUM") as xtp,
            tc.tile_pool(name="xts", bufs=16) as xts,
            tc.tile_pool(name="pop", bufs=4, space="PSUM") as pop,
            tc.tile_pool(name="outp", bufs=3) as outp,
        ):
            # Resident weights: per segment, u-chunk tiles [128, n-cols].
            # f32r matmul inputs must be produced rounded to f32r, so stage
            # the f32 DMA and round via a DVE copy.
            w_sb = []
            for si, (mul, ird, off, n_uc) in enumerate(SEGS):
                ncols = mm_cols[si]
                chunks = []
                for uc in range(n_uc):
                    t = wpool.tile([P, ncols], mm_dt, tag=f"w{si}_{uc}")
                    if use_f32r:
                        stg = wpool.tile([P, ncols], f32, tag=f"wstg{si}_{uc}")
                        nc.sync.dma_start(
                            out=stg[:], in_=w_dram[si][uc * P:(uc + 1) * P, :ncols]
                        )
                        nc.vector.tensor_copy(out=t[:], in_=stg[:])
                    else:
                        nc.sync.dma_start(
                            out=t[:], in_=w_dram[si][uc * P:(uc + 1) * P, :ncols]
                        )
                    chunks.append(t)
                w_sb.append(chunks)
            ident_sb = wpool.tile([P, P], f32, tag="ident")
            nc.sync.dma_start(out=ident_sb[:], in_=ident[:, :])

            for bt in range(BT):
                r0 = bt * P
                xt = xin.tile([P, X_DIM], f32, tag="x")
                nc.sync.dma_start(out=xt[:], in_=x[r0:r0 + P, :])
                ot = outp.tile([P, X_DIM], f32, tag="o")

                for si, (mul, ird, off, n_uc) in enumerate(SEGS):
                    ncols = mm_cols[si]
                    for i in range(ird):
                        xs_chunks = []
                        for uc in range(n_uc):
                            start = off + uc * P * ird + i
                            src = xt[:, start:start + ird * (P - 1) + 1:ird]
                            tp = xtp.tile([P, P], f32, tag="tp")
                            nc.tensor.transpose(tp[:], src, ident_sb[:])
                            xs = xts.tile([P, P], mm_dt, tag="xs")
                            nc.vector.tensor_copy(out=xs[:], in_=tp[:])
                            xs_chunks.append(xs)
                        po = pop.tile([P, ncols], f32, tag="po")
                        for uc in range(n_uc):
                            nc.tensor.matmul(
                                po[:],
                                xs_chunks[uc][:],
                                w_sb[si][uc][:],
                                start=(uc == 0),
                                stop=(uc == n_uc - 1),
                            )
                        dst = ot[:, off + i:off + i + ird * (mul - 1) + 1:ird]
                        nc.vector.tensor_copy(out=dst, in_=po[:, :mul])

                nc.sync.dma_start(out=y[r0:r0 + P, :], in_=ot[:])

    nc.compile()
    return nc


def _get_program(mm_dtype_name: str):
    if mm_dtype_name not in _cache:
        _cache[mm_dtype_name] = _build_program(mm_dtype_name)
    return _cache[mm_dtype_name]


def kernel(x: np.ndarray, weight: np.ndarray, _mm_dtype: str = "float32",
           _trace: bool = False):
    from concourse.bass_utils import run_bass_kernel_spmd

    nc = _get_program(_mm_dtype)

    x = np.ascontiguousarray(np.asarray(x, dtype=np.float32))
    weight = np.asarray(weight, dtype=np.float32)

    w1 = (weight[:65536].reshape(256, 256) / np.sqrt(np.float32(256.0))).astype(np.float32)
    w2 = (weight[65536:131072].reshape(256, 256) / np.sqrt(np.float32(256.0))).astype(np.float32)
    w3 = np.zeros((128, 256), dtype=np.float32)
    w3[:, :128] = weight[131072:].reshape(128, 128) / np.sqrt(np.float32(128.0))
    ident = np.eye(P, dtype=np.float32)

    xp = np.zeros((N_CORES * ROWS_PER_CORE, X_DIM), dtype=np.float32)
    xp[:BATCH] = x

    in_maps = [
        {
            "x": xp[c * ROWS_PER_CORE:(c + 1) * ROWS_PER_CORE],
            "w1": w1,
            "w2": w2,
            "w3": w3,
            "ident": ident,
        }
        for c in range(N_CORES)
    ]
    res = run_bass_kernel_spmd(
        nc, in_maps, list(range(N_CORES)), trace=_trace
    )
    out = np.concatenate([res.results[c]["y"] for c in range(N_CORES)], axis=0)
    if _trace:
        kernel.last_exec_time_ns = res.exec_time_ns
    return out[:BATCH]


# revision 11
# speedup vs baseline: 1.3959x; 1.1609x over previous
"""Equivariant block-diagonal linear (irreps 256x0e + 256x1o + 128x2e) on 8
Trainium2 NeuronCores.

Math: for each irrep segment (mul, ird), out[b, v, i] = c * sum_u w[u,v] *
x[b, u, i] with c = 1/sqrt(mul). x columns are laid out mul-major:
col = seg_off + u*ird + i.

Data-parallel sharding: batch 50000 -> 8 cores x 6272 rows (padded).
Per core, per 128-row batch tile:
  1. DMA x rows to SBUF (256-row superloads for DMA efficiency).
  2. For each (segment, i, u-chunk): PE-transpose the strided column slice
     x[:, off+i::ird] (128 u values) into a shared PSUM tile (two
     transposes per tile) -> one DVE cast [128,256] to SBUF f32r.
  3. matmul(psum[b, v] += xT[u_chunk].T @ w[u_chunk, :]): seg1/seg2 use
     f32r (1 cycle/row at N=256), seg3 uses fp32 (N=128 would fall off
     the f32r fast path). Per segment the per-i matmuls write disjoint
     slices of one PSUM region, each slice within a single PSUM bank.
  4. One DVE copy per segment de-interleaves PSUM -> the [128, 1664]
     output staging tile (strided dst). One DMA per 256 rows to DRAM.

Weights are host-prescaled by c and stay SBUF-resident in natural [u, v]
layout (f32r copies for seg1/2).
"""

import numpy as np

N_CORES = 8
BATCH = 50000
X_DIM = 1664
P = 128
ROWS_PER_CORE = 6272  # 49 tiles of 128; 8*6272 = 50176 >= 50000
BT = ROWS_PER_CORE // P

# (mul, ird, x/y col offset, n u-chunks of 128)
SEGS = [
    (256, 1, 0, 2),
    (256, 3, 256, 2),
    (128, 5, 1024, 1),
]

_cache = {}


def _build_program(mode: str):
    import concourse.bacc as bacc
    import concourse.mybir as mybir
    from concourse.tile import TileContext

    use_f32r = mode == "float32r"
    f32 = mybir.dt.float32
    f32r = mybir.dt.float32r
    mm_dt = f32r if use_f32r else f32

    nc = bacc.Bacc(
        "TRN2", target_bir_lowering=False, debug=False, num_devices=N_CORES
    )
    x = nc.dram_tensor("x", [ROWS_PER_CORE, X_DIM], f32, kind="ExternalInput")
    w1 = nc.dram_tensor("w1", [256, 256], f32, kind="ExternalInput")
    w2 = nc.dram_tensor("w2", [256, 256], f32, kind="ExternalInput")
    w3 = nc.dram_tensor("w3", [128, 128], f32, kind="ExternalInput")
    ident = nc.dram_tensor("ident", [P, P], f32, kind="ExternalInput")
    y = nc.dram_tensor("y", [ROWS_PER_CORE, X_DIM], f32, kind="ExternalOutput")
    w_dram = [w1, w2, w3]
    # seg3 runs fp32 (N=128 misses the f32r >=256 fast path anyway).
    seg_dt = [mm_dt, mm_dt, f32]

    with TileContext(nc) as tc:
        with (
            tc.tile_pool(name="wpool", bufs=1) as wpool,
            tc.tile_pool(name="xin", bufs=3) as xin,
            tc.tile_pool(name="xtp", bufs=3, space="PSUM") as xtp,
            tc.tile_pool(name="xts", bufs=10) as xts,
            tc.tile_pool(name="po1p", bufs=1, space="PSUM") as po1p,
            tc.tile_pool(name="po2p", bufs=1, space="PSUM") as po2p,
            tc.tile_pool(name="po3p", bufs=1, space="PSUM") as po3p,
            tc.tile_pool(name="outp", bufs=3) as outp,
        ):
            # Resident weights, natural [u, v] layout, one [128, mul] chunk
            # per 128 u's. f32r inputs must be produced rounded, hence the
            # staged DVE copy.
            w_sb = []
            for si, (mul, ird, off, n_uc) in enumerate(SEGS):
                chunks = []
                for uc in range(n_uc):
                    t = wpool.tile([P, mul], seg_dt[si], tag=f"w{si}_{uc}")
                    if seg_dt[si] == f32:
                        nc.sync.dma_start(
                            out=t[:], in_=w_dram[si][uc * P:(uc + 1) * P, :]
                        )
                    else:
                        stg = wpool.tile([P, mul], f32, tag=f"wstg{si}_{uc}")
                        nc.sync.dma_start(
                            out=stg[:], in_=w_dram[si][uc * P:(uc + 1) * P, :]
                        )
                        nc.vector.tensor_copy(out=t[:], in_=stg[:])
                    chunks.append(t)
                w_sb.append(chunks)
            ident_sb = wpool.tile([P, P], f32, tag="ident")
            nc.sync.dma_start(out=ident_sb[:], in_=ident[:, :])

            # Batch loop: pairs of 128-row tiles share one DMA (1.7 MB
            # transfers), with a single-tile epilogue if BT is odd.
            groups = [(g * 2, 2) for g in range(BT // 2)]
            if BT % 2:
                groups.append((BT - 1, 1))

            for bt0, ntile in groups:
                r0 = bt0 * P
                xt = xin.tile([P, 2 * X_DIM], f32, tag="x")
                nc.sync.dma_start(
                    out=xt[:, :ntile * X_DIM].rearrange(
                        "p (t c) -> p t c", t=ntile
                    ),
                    in_=x[r0:r0 + ntile * P, :].rearrange(
                        "(t p) c -> p t c", p=P
                    ),
                )
                ot = outp.tile([P, 2 * X_DIM], f32, tag="o")

                for t in range(ntile):
                    xoff = t * X_DIM

                    # --- transposes + casts: (seg, i, uc) -> xs tiles ---
                    # xs_map[(si, i, uc)] = (tile, col0)
                    xs_map = {}
                    pend = []  # pending halves in current tp/xs pair

                    def flush(pend):
                        if not pend:
                            return
                        width = P * len(pend)
                        dt_ = pend[0][3]
                        tp = xtp.tile([P, 2 * P], f32, tag="tp")
                        for h, (si, i, uc, _d, src) in enumerate(pend):
                            nc.tensor.transpose(
                                tp[:, h * P:(h + 1) * P], src, ident_sb[:]
                            )
                        xs = xts.tile([P, 2 * P], dt_, tag="xs")
                        nc.vector.tensor_copy(
                            out=xs[:, :width], in_=tp[:, :width]
                        )
                        for h, (si, i, uc, _d, src) in enumerate(pend):
                            xs_map[(si, i, uc)] = (xs, h * P)
                        pend.clear()

                    for si, (mul, ird, off, n_uc) in enumerate(SEGS):
                        for i in range(ird):
                            for uc in range(n_uc):
                                start = xoff + off + uc * P * ird + i
                                src = xt[:, start:start + ird * (P - 1) + 1:ird]
                                if pend and pend[0][3] != seg_dt[si]:
                                    flush(pend)
                                pend.append((si, i, uc, seg_dt[si], src))
                                if len(pend) == 2:
                                    flush(pend)
                    flush(pend)

                    # --- matmuls into per-segment PSUM regions ---
                    po1 = po1p.tile([P, 256], f32, tag="po1")
                    po2 = po2p.tile([P, 768], f32, tag="po2")
                    po3 = po3p.tile([P, 640], f32, tag="po3")
                    pos = [po1, po2, po3]
                    for si, (mul, ird, off, n_uc) in enumerate(SEGS):
                        for i in range(ird):
                            dst = pos[si][:, i * mul:(i + 1) * mul]
                            for uc in range(n_uc):
                                xs, c0 = xs_map[(si, i, uc)]
                                nc.tensor.matmul(
                                    dst,
                                    xs[:, c0:c0 + P],
                                    w_sb[si][uc][:],
                                    start=(uc == 0),
                                    stop=(uc == n_uc - 1),
                                )

                    # --- de-interleave PSUM -> output staging ---
                    for si, (mul, ird, off, n_uc) in enumerate(SEGS):
                        seg_w = mul * ird
                        src = pos[si][:].rearrange("p (i v) -> p i v", i=ird)
                        dst = ot[
                            :, xoff + off:xoff + off + seg_w
                        ].rearrange("p (v i) -> p i v", i=ird)
                        nc.vector.tensor_copy(out=dst, in_=src)

                nc.sync.dma_start(
                    out=y[r0:r0 + ntile * P, :].rearrange(
                        "(t p) c -> p t c", p=P
                    ),
                    in_=ot[:, :ntile * X_DIM].rearrange(
                        "p (t c) -> p t c", t=ntile
                    ),
                )

    nc.compile()
    return nc


def _get_program(mode: str):
    if mode not in _cache:
        _cache[mode] = _build_program(mode)
    return _cache[mode]


def kernel(x: np.ndarray, weight: np.ndarray, _mm_dtype: str = "float32r",
           _trace: bool = False):
    from concourse.bass_utils import run_bass_kernel_spmd

    nc = _get_program(_mm_dtype)

    x = np.ascontiguousarray(np.asarray(x, dtype=np.float32))
    weight = np.asarray(weight, dtype=np.float32)

    w1 = (weight[:65536].reshape(256, 256) / np.sqrt(np.float32(256.0))).astype(np.float32)
    w2 = (weight[65536:131072].reshape(256, 256) / np.sqrt(np.float32(256.0))).astype(np.float32)
    w3 = (weight[131072:].reshape(128, 128) / np.sqrt(np.float32(128.0))).astype(np.float32)
    ident = np.eye(P, dtype=np.float32)

    xp = np.zeros((N_CORES * ROWS_PER_CORE, X_DIM), dtype=np.float32)
    xp[:BATCH] = x

    in_maps = [
        {
            "x": xp[c * ROWS_PER_CORE:(c + 1) * ROWS_PER_CORE],
            "w1": w1,
            "w2": w2,
            "w3": w3,
            "ident": ident,
        }
        for c in range(N_CORES)
    ]
    res = run_bass_kernel_spmd(
        nc, in_maps, list(range(N_CORES)), trace=_trace
    )
    out = np.concatenate([res.results[c]["y"] for c in range(N_CORES)], axis=0)
    if _trace:
        kernel.last_exec_time_ns = res.exec_time_ns
    return out[:BATCH]


# revision 12
# speedup vs baseline: 1.4004x; 1.0032x over previous
"""Equivariant block-diagonal linear (irreps 256x0e + 256x1o + 128x2e) on 8
Trainium2 NeuronCores.

Math: for each irrep segment (mul, ird), out[b, v, i] = c * sum_u w[u,v] *
x[b, u, i] with c = 1/sqrt(mul). x columns are laid out mul-major:
col = seg_off + u*ird + i.

Data-parallel sharding: batch 50000 -> 8 cores x 6272 rows (padded).
Per core, per 128-row batch tile:
  1. DMA x rows to SBUF (256-row superloads for DMA efficiency).
  2. For each (segment, i, u-chunk): PE-transpose the strided column slice
     x[:, off+i::ird] (128 u values) into a shared PSUM tile (two
     transposes per tile) -> one DVE cast [128,256] to SBUF f32r.
  3. matmul(psum[b, v] += xT[u_chunk].T @ w[u_chunk, :]): seg1/seg2 use
     f32r (1 cycle/row at N=256), seg3 uses fp32 (N=128 would fall off
     the f32r fast path). Per segment the per-i matmuls write disjoint
     slices of one PSUM region, each slice within a single PSUM bank.
  4. One DVE copy per segment de-interleaves PSUM -> the [128, 1664]
     output staging tile (strided dst). One DMA per 256 rows to DRAM.

Weights are host-prescaled by c and stay SBUF-resident in natural [u, v]
layout (f32r copies for seg1/2).
"""

import numpy as np

N_CORES = 8
BATCH = 50000
X_DIM = 1664
P = 128
ROWS_PER_CORE = 6272  # 49 tiles of 128; 8*6272 = 50176 >= 50000
BT = ROWS_PER_CORE // P

# (mul, ird, x/y col offset, n u-chunks of 128)
SEGS = [
    (256, 1, 0, 2),
    (256, 3, 256, 2),
    (128, 5, 1024, 1),
]

_cache = {}


def _build_program(mode: str):
    import concourse.bacc as bacc
    import concourse.mybir as mybir
    from concourse.tile import TileContext

    use_f32r = mode == "float32r"
    f32 = mybir.dt.float32
    f32r = mybir.dt.float32r
    mm_dt = f32r if use_f32r else f32

    nc = bacc.Bacc(
        "TRN2", target_bir_lowering=False, debug=False, num_devices=N_CORES
    )
    x = nc.dram_tensor("x", [ROWS_PER_CORE, X_DIM], f32, kind="ExternalInput")
    w1 = nc.dram_tensor("w1", [256, 256], f32, kind="ExternalInput")
    w2 = nc.dram_tensor("w2", [256, 256], f32, kind="ExternalInput")
    w3 = nc.dram_tensor("w3", [128, 128], f32, kind="ExternalInput")
    ident = nc.dram_tensor("ident", [P, P], f32, kind="ExternalInput")
    y = nc.dram_tensor("y", [ROWS_PER_CORE, X_DIM], f32, kind="ExternalOutput")
    w_dram = [w1, w2, w3]
    # seg3 runs fp32 (N=128 misses the f32r >=256 fast path anyway).
    seg_dt = [mm_dt, mm_dt, f32]

    with TileContext(nc) as tc:
        with (
            tc.tile_pool(name="wpool", bufs=1) as wpool,
            tc.tile_pool(name="xin", bufs=3) as xin,
            tc.tile_pool(name="xtp", bufs=3, space="PSUM") as xtp,
            tc.tile_pool(name="xts", bufs=10) as xts,
            tc.tile_pool(name="po1p", bufs=1, space="PSUM") as po1p,
            tc.tile_pool(name="po2p", bufs=1, space="PSUM") as po2p,
            tc.tile_pool(name="po3p", bufs=1, space="PSUM") as po3p,
            tc.tile_pool(name="outp", bufs=3) as outp,
        ):
            # Resident weights, natural [u, v] layout, one [128, mul] chunk
            # per 128 u's. f32r inputs must be produced rounded, hence the
            # staged DVE copy.
            w_sb = []
            for si, (mul, ird, off, n_uc) in enumerate(SEGS):
                chunks = []
                for uc in range(n_uc):
                    t = wpool.tile([P, mul], seg_dt[si], tag=f"w{si}_{uc}")
                    if seg_dt[si] == f32:
                        nc.sync.dma_start(
                            out=t[:], in_=w_dram[si][uc * P:(uc + 1) * P, :]
                        )
                    else:
                        stg = wpool.tile([P, mul], f32, tag=f"wstg{si}_{uc}")
                        nc.sync.dma_start(
                            out=stg[:], in_=w_dram[si][uc * P:(uc + 1) * P, :]
                        )
                        nc.vector.tensor_copy(out=t[:], in_=stg[:])
                    chunks.append(t)
                w_sb.append(chunks)
            ident_sb = wpool.tile([P, P], f32, tag="ident")
            nc.sync.dma_start(out=ident_sb[:], in_=ident[:, :])

            # Batch loop: pairs of 128-row tiles share one DMA (1.7 MB
            # transfers), with a single-tile epilogue if BT is odd.
            groups = [(g * 2, 2) for g in range(BT // 2)]
            if BT % 2:
                groups.append((BT - 1, 1))

            for bt0, ntile in groups:
                r0 = bt0 * P
                xt = xin.tile([P, 2 * X_DIM], f32, tag="x")
                nc.sync.dma_start(
                    out=xt[:, :ntile * X_DIM].rearrange(
                        "p (t c) -> p t c", t=ntile
                    ),
                    in_=x[r0:r0 + ntile * P, :].rearrange(
                        "(t p) c -> p t c", p=P
                    ),
                )
                ot = outp.tile([P, 2 * X_DIM], f32, tag="o")

                for t in range(ntile):
                    xoff = t * X_DIM

                    # --- transposes + casts: (seg, i, uc) -> xs tiles ---
                    # xs_map[(si, i, uc)] = (tile, col0)
                    xs_map = {}
                    pend = []  # pending halves in current tp/xs pair

                    def flush(pend):
                        if not pend:
                            return
                        width = P * len(pend)
                        dt_ = pend[0][3]
                        tp = xtp.tile([P, 2 * P], f32, tag="tp")
                        for h, (si, i, uc, _d, src) in enumerate(pend):
                            nc.tensor.transpose(
                                tp[:, h * P:(h + 1) * P], src, ident_sb[:]
                            )
                        xs = xts.tile([P, 2 * P], dt_, tag="xs")
                        nc.vector.tensor_copy(
                            out=xs[:, :width], in_=tp[:, :width]
                        )
                        for h, (si, i, uc, _d, src) in enumerate(pend):
                            xs_map[(si, i, uc)] = (xs, h * P)
                        pend.clear()

                    for si, (mul, ird, off, n_uc) in enumerate(SEGS):
                        for i in range(ird):
                            for uc in range(n_uc):
                                start = xoff + off + uc * P * ird + i
                                src = xt[:, start:start + ird * (P - 1) + 1:ird]
                                if pend and pend[0][3] != seg_dt[si]:
                                    flush(pend)
                                pend.append((si, i, uc, seg_dt[si], src))
                                if len(pend) == 2:
                                    flush(pend)
                    flush(pend)

                    # --- matmuls into per-segment PSUM regions ---
                    po1 = po1p.tile([P, 256], f32, tag="po1")
                    po2 = po2p.tile([P, 768], f32, tag="po2")
                    po3 = po3p.tile([P, 640], f32, tag="po3")
                    pos = [po1, po2, po3]
                    for si, (mul, ird, off, n_uc) in enumerate(SEGS):
                        for i in range(ird):
                            dst = pos[si][:, i * mul:(i + 1) * mul]
                            for uc in range(n_uc):
                                xs, c0 = xs_map[(si, i, uc)]
                                nc.tensor.matmul(
                                    dst,
                                    xs[:, c0:c0 + P],
                                    w_sb[si][uc][:],
                                    start=(uc == 0),
                                    stop=(uc == n_uc - 1),
                                )

                    # --- de-interleave PSUM -> output staging ---
                    # Strided-dst copies are slow; the scalar engine is idle,
                    # so it takes the two interleaved segments while DVE
                    # keeps the contiguous seg1 copy (plus casts above).
                    for si, (mul, ird, off, n_uc) in enumerate(SEGS):
                        seg_w = mul * ird
                        src = pos[si][:].rearrange("p (i v) -> p i v", i=ird)
                        dst = ot[
                            :, xoff + off:xoff + off + seg_w
                        ].rearrange("p (v i) -> p i v", i=ird)
                        if si == 0:
                            nc.vector.tensor_copy(out=dst, in_=src)
                        else:
                            nc.scalar.copy(out=dst, in_=src)

                nc.sync.dma_start(
                    out=y[r0:r0 + ntile * P, :].rearrange(
                        "(t p) c -> p t c", p=P
                    ),
                    in_=ot[:, :ntile * X_DIM].rearrange(
                        "p (t c) -> p t c", t=ntile
                    ),
                )

    nc.compile()
    return nc


def _get_program(mode: str):
    if mode not in _cache:
        _cache[mode] = _build_program(mode)
    return _cache[mode]


def kernel(x: np.ndarray, weight: np.ndarray, _mm_dtype: str = "float32r",
           _trace: bool = False):
    from concourse.bass_utils import run_bass_kernel_spmd

    nc = _get_program(_mm_dtype)

    x = np.ascontiguousarray(np.asarray(x, dtype=np.float32))
    weight = np.asarray(weight, dtype=np.float32)

    w1 = (weight[:65536].reshape(256, 256) / np.sqrt(np.float32(256.0))).astype(np.float32)
    w2 = (weight[65536:131072].reshape(256, 256) / np.sqrt(np.float32(256.0))).astype(np.float32)
    w3 = (weight[131072:].reshape(128, 128) / np.sqrt(np.float32(128.0))).astype(np.float32)
    ident = np.eye(P, dtype=np.float32)

    xp = np.zeros((N_CORES * ROWS_PER_CORE, X_DIM), dtype=np.float32)
    xp[:BATCH] = x

    in_maps = [
        {
            "x": xp[c * ROWS_PER_CORE:(c + 1) * ROWS_PER_CORE],
            "w1": w1,
            "w2": w2,
            "w3": w3,
            "ident": ident,
        }
        for c in range(N_CORES)
    ]
    res = run_bass_kernel_spmd(
        nc, in_maps, list(range(N_CORES)), trace=_trace
    )
    out = np.concatenate([res.results[c]["y"] for c in range(N_CORES)], axis=0)
    if _trace:
        kernel.last_exec_time_ns = res.exec_time_ns
    return out[:BATCH]


# revision 15
# speedup vs baseline: 1.5948x; 1.1388x over previous
"""Equivariant block-diagonal linear (irreps 256x0e + 256x1o + 128x2e) on 8
Trainium2 NeuronCores.

Math: for each irrep segment (mul, ird), out[b, v, i] = c * sum_u w[u,v] *
x[b, u, i] with c = 1/sqrt(mul). x columns are laid out mul-major:
col = seg_off + u*ird + i.

Data-parallel sharding: batch 50000 -> 8 cores x 6272 rows (padded).
Per core, per 128-row batch tile:
  1. DMA x rows to SBUF (256-row superloads for DMA efficiency).
  2. For each (segment, i, u-chunk): PE-transpose the strided column slice
     x[:, off+i::ird] (128 u values) into a shared PSUM tile (two
     transposes per tile) -> one DVE cast [128,256] to SBUF f32r.
  3. matmul(psum[b, v] += xT[u_chunk].T @ w[u_chunk, :]): seg1/seg2 use
     f32r (1 cycle/row at N=256), seg3 uses fp32 (N=128 would fall off
     the f32r fast path). Per segment the per-i matmuls write disjoint
     slices of one PSUM region, each slice within a single PSUM bank.
  4. One DVE copy per segment de-interleaves PSUM -> the [128, 1664]
     output staging tile (strided dst). One DMA per 256 rows to DRAM.

Weights are host-prescaled by c and stay SBUF-resident in natural [u, v]
layout (f32r copies for seg1/2).
"""

import numpy as np

N_CORES = 8
BATCH = 50000
X_DIM = 1664
P = 128
ROWS_PER_CORE = 6272  # 49 tiles of 128; 8*6272 = 50176 >= 50000
BT = ROWS_PER_CORE // P

# (mul, ird, x/y col offset, n u-chunks of 128)
SEGS = [
    (256, 1, 0, 2),
    (256, 3, 256, 2),
    (128, 5, 1024, 1),
]

_cache = {}


def _build_program(mode: str):
    import concourse.bacc as bacc
    import concourse.mybir as mybir
    from concourse.tile import TileContext

    use_f32r = mode == "float32r"
    f32 = mybir.dt.float32
    f32r = mybir.dt.float32r
    mm_dt = f32r if use_f32r else f32

    nc = bacc.Bacc(
        "TRN2", target_bir_lowering=False, debug=False, num_devices=N_CORES
    )
    x = nc.dram_tensor("x", [ROWS_PER_CORE, X_DIM], f32, kind="ExternalInput")
    w1 = nc.dram_tensor("w1", [256, 256], f32, kind="ExternalInput")
    w2 = nc.dram_tensor("w2", [256, 256], f32, kind="ExternalInput")
    w3 = nc.dram_tensor("w3", [128, 128], f32, kind="ExternalInput")
    ident = nc.dram_tensor("ident", [P, P], f32, kind="ExternalInput")
    y = nc.dram_tensor("y", [ROWS_PER_CORE, X_DIM], f32, kind="ExternalOutput")
    w_dram = [w1, w2, w3]
    # seg3 runs fp32 (N=128 misses the f32r >=256 fast path anyway).
    seg_dt = [mm_dt, mm_dt, f32]

    with TileContext(nc) as tc:
        with (
            tc.tile_pool(name="wpool", bufs=1) as wpool,
            tc.tile_pool(name="xin", bufs=4) as xin,
            tc.tile_pool(name="xtp", bufs=3, space="PSUM") as xtp,
            tc.tile_pool(name="xts", bufs=10) as xts,
            tc.tile_pool(name="po1p", bufs=1, space="PSUM") as po1p,
            tc.tile_pool(name="po2p", bufs=1, space="PSUM") as po2p,
            tc.tile_pool(name="po3p", bufs=1, space="PSUM") as po3p,
            tc.tile_pool(name="outp", bufs=4) as outp,
        ):
            # Resident weights, natural [u, v] layout, one [128, mul] chunk
            # per 128 u's. f32r inputs must be produced rounded, hence the
            # staged DVE copy.
            w_sb = []
            for si, (mul, ird, off, n_uc) in enumerate(SEGS):
                chunks = []
                for uc in range(n_uc):
                    t = wpool.tile([P, mul], seg_dt[si], tag=f"w{si}_{uc}")
                    if seg_dt[si] == f32:
                        nc.sync.dma_start(
                            out=t[:], in_=w_dram[si][uc * P:(uc + 1) * P, :]
                        )
                    else:
                        stg = wpool.tile([P, mul], f32, tag=f"wstg{si}_{uc}")
                        nc.sync.dma_start(
                            out=stg[:], in_=w_dram[si][uc * P:(uc + 1) * P, :]
                        )
                        nc.vector.tensor_copy(out=t[:], in_=stg[:])
                    chunks.append(t)
                w_sb.append(chunks)
            ident_sb = wpool.tile([P, P], f32, tag="ident")
            nc.sync.dma_start(out=ident_sb[:], in_=ident[:, :])

            # Batch loop: pairs of 128-row tiles share one DMA (1.7 MB
            # transfers), with a single-tile epilogue if BT is odd.
            groups = [(g * 2, 2) for g in range(BT // 2)]
            if BT % 2:
                groups.append((BT - 1, 1))

            for bt0, ntile in groups:
                r0 = bt0 * P
                xt = xin.tile([P, 2 * X_DIM], f32, tag="x")
                nc.sync.dma_start(
                    out=xt[:, :ntile * X_DIM].rearrange(
                        "p (t c) -> p t c", t=ntile
                    ),
                    in_=x[r0:r0 + ntile * P, :].rearrange(
                        "(t p) c -> p t c", p=P
                    ),
                )
                ot = outp.tile([P, 2 * X_DIM], f32, tag="o")

                for t in range(ntile):
                    xoff = t * X_DIM

                    # --- transposes + casts: (seg, i, uc) -> xs tiles ---
                    # xs_map[(si, i, uc)] = (tile, col0)
                    xs_map = {}
                    pend = []  # pending halves in current tp/xs pair

                    def flush(pend):
                        if not pend:
                            return
                        width = P * len(pend)
                        dt_ = pend[0][3]
                        tp = xtp.tile([P, 2 * P], f32, tag="tp")
                        for h, (si, i, uc, _d, src) in enumerate(pend):
                            nc.tensor.transpose(
                                tp[:, h * P:(h + 1) * P], src, ident_sb[:]
                            )
                        xs = xts.tile([P, 2 * P], dt_, tag="xs")
                        nc.vector.tensor_copy(
                            out=xs[:, :width], in_=tp[:, :width]
                        )
                        for h, (si, i, uc, _d, src) in enumerate(pend):
                            xs_map[(si, i, uc)] = (xs, h * P)
                        pend.clear()

                    for si, (mul, ird, off, n_uc) in enumerate(SEGS):
                        for i in range(ird):
                            for uc in range(n_uc):
                                start = xoff + off + uc * P * ird + i
                                src = xt[:, start:start + ird * (P - 1) + 1:ird]
                                if pend and pend[0][3] != seg_dt[si]:
                                    flush(pend)
                                pend.append((si, i, uc, seg_dt[si], src))
                                if len(pend) == 2:
                                    flush(pend)
                    flush(pend)

                    # --- matmuls into per-segment PSUM regions ---
                    po1 = po1p.tile([P, 256], f32, tag="po1")
                    po2 = po2p.tile([P, 768], f32, tag="po2")
                    po3 = po3p.tile([P, 640], f32, tag="po3")
                    pos = [po1, po2, po3]
                    for si, (mul, ird, off, n_uc) in enumerate(SEGS):
                        for i in range(ird):
                            dst = pos[si][:, i * mul:(i + 1) * mul]
                            for uc in range(n_uc):
                                xs, c0 = xs_map[(si, i, uc)]
                                nc.tensor.matmul(
                                    dst,
                                    xs[:, c0:c0 + P],
                                    w_sb[si][uc][:],
                                    start=(uc == 0),
                                    stop=(uc == n_uc - 1),
                                )

                    # --- de-interleave PSUM -> output staging ---
                    # Strided-dst copies are slow; the scalar engine is idle,
                    # so it takes the two interleaved segments while DVE
                    # keeps the contiguous seg1 copy (plus casts above).
                    for si, (mul, ird, off, n_uc) in enumerate(SEGS):
                        seg_w = mul * ird
                        src = pos[si][:].rearrange("p (i v) -> p i v", i=ird)
                        dst = ot[
                            :, xoff + off:xoff + off + seg_w
                        ].rearrange("p (v i) -> p i v", i=ird)
                        if si == 0:
                            nc.vector.tensor_copy(out=dst, in_=src)
                        else:
                            nc.scalar.copy(out=dst, in_=src)

                # Output stores ride the ACT HWDGE ring so input prefetch
                # (SP ring) never queues behind them.
                nc.scalar.dma_start(
                    out=y[r0:r0 + ntile * P, :].rearrange(
                        "(t p) c -> p t c", p=P
                    ),
                    in_=ot[:, :ntile * X_DIM].rearrange(
                        "p (t c) -> p t c", t=ntile
                    ),
                )

    nc.compile()
    return nc


def _get_program(mode: str):
    if mode not in _cache:
        _cache[mode] = _build_program(mode)
    return _cache[mode]


def kernel(x: np.ndarray, weight: np.ndarray, _mm_dtype: str = "float32r",
           _trace: bool = False):
    from concourse.bass_utils import run_bass_kernel_spmd

    nc = _get_program(_mm_dtype)

    x = np.ascontiguousarray(np.asarray(x, dtype=np.float32))
    weight = np.asarray(weight, dtype=np.float32)

    w1 = (weight[:65536].reshape(256, 256) / np.sqrt(np.float32(256.0))).astype(np.float32)
    w2 = (weight[65536:131072].reshape(256, 256) / np.sqrt(np.float32(256.0))).astype(np.float32)
    w3 = (weight[131072:].reshape(128, 128) / np.sqrt(np.float32(128.0))).astype(np.float32)
    ident = np.eye(P, dtype=np.float32)

    xp = np.zeros((N_CORES * ROWS_PER_CORE, X_DIM), dtype=np.float32)
    xp[:BATCH] = x

    in_maps = [
        {
            "x": xp[c * ROWS_PER_CORE:(c + 1) * ROWS_PER_CORE],
            "w1": w1,
            "w2": w2,
            "w3": w3,
            "ident": ident,
        }
        for c in range(N_CORES)
    ]
    res = run_bass_kernel_spmd(
        nc, in_maps, list(range(N_CORES)), trace=_trace
    )
    out = np.concatenate([res.results[c]["y"] for c in range(N_CORES)], axis=0)
    if _trace:
        kernel.last_exec_time_ns = res.exec_time_ns
    return out[:BATCH]
